# revision 9
# baseline (speedup 1.0000x reference)
"""Trainium2 Bass kernel for: out = (x @ wsums.sum(0)) * (1.5 * 0.5).

x: [1024, 8192] f32, wsums: [32, 8192] f32 -> out: [1024, 1] f32.

Sharding across 8 NeuronCores: 8-way along the contraction dim k
(8192 -> 1024 per core).  Each core reads a 4MB x column-shard plus its
128KB wsums k-slice, computes partial dot products for ALL 1024 rows over
its k-slice, and the host sums the 8 per-core partials (the unshard step
for a contraction-sharded dim).

v3 design (from v1 29.4-30.2us, v2 33.3us):
  - x and wsums are DMA'd through the SWDGE (gpsimd) path with an f32->bf16
    cast in the SDMA datapath.  HBM read bytes are unchanged (the memory
    bound) and v2 measured the same ~430GB/s read rate as HWDGE, but all
    tensor_tensor multiplies then run at the DVE 2x bf16 packed rate.
  - SDMA engine 15 is reliably slow (known trn2 erratum, engines 7/15): in
    v1/v2 traces it starts ~2us late and stalls ~1.5us, pacing every
    chunk's completion semaphore (+3us on the worst core).  Fix: home
    DMAs cover only partitions 0..123, so engine 15 (partitions 92-95 +
    124-127) carries half the bytes; rows 124..127 of all 8 row-blocks
    are batched into one [32, KB] spill tile on partitions 0-31 (one DMA,
    one TT, one accum), and the host folds the spill partials back in.
  - The per-element accumulation pass (~1 elem/cycle on every engine,
    dtype-independent - v2 measured DVE reduce 1.21us and ScalarE ACT
    1.43us per 1024-wide block) is the wall, so it is spread over THREE
    engines: ScalarE activation accum_out, DVE tensor_reduce, and Pool
    tensor_reduce, per the ACCUM_PLAN table.
  - Block 0 is processed last as two half-k pieces; acc columns 0..7 are
    stored while the last half-block computes; only a 124x4B store +
    completion receipt trails the final reduce.

Environment workarounds (this container's walrus build):
  - it encodes at most ONE semaphore wait per instruction ("Too many sync
    wait commands"), so compile_bir_kernel is wrapped with a BIR post-pass
    that moves excess waits onto preceding same-engine NoOp instructions;
  - it cannot encode bass_isa raw-ISA ops (tensor_tensor_reduce,
    partition_all_reduce, ... -> "ISA wrong length"), so only classic
    mybir ops are used.
"""

import json

import numpy as np

import concourse.bass as bass
import concourse.bass2jax as bass2jax
import concourse.bass_utils as bass_utils
import concourse.mybir as mybir
from concourse.tile import TileContext

SCALE = 1.5 * 0.5
B, K, G = 1024, 8192, 32
N_CORES = 8
KSHARD = 8                  # cores along k
BSHARD = N_CORES // KSHARD  # cores along batch
KB = K // KSHARD            # per-core k width
BB = B // BSHARD            # per-core rows
P = 128
HOME_P = 124                # home partitions (engine 15 serves 92-95,124-127)
SPILL_ROWS = P - HOME_P     # rows per block in the spill tile
NBLK = BB // P              # row-blocks per core
NCOL = NBLK + 1             # acc columns: blk1..7 -> 0..6, blk0 lo/hi -> 7/8
NSP = SPILL_ROWS * NBLK     # spill tile partitions
F32 = mybir.dt.float32
BF16 = mybir.dt.bfloat16

# Set by test.py to profile; results stashed in LAST_RESULTS.
TRACE = False
TRACE_KWARGS = {}
LAST_RESULTS = None

_built = None

# Accumulating engine per piece (v2 measured: ScalarE ACT 1.43us/full
# block incl accumulator read, DVE reduce 1.21us; Pool tensor_reduce can't
# reduce the free axis, so "pool" pieces run a 3-level tensor_add tree on
# Pool (1024 -> 128 wide, f32 partials, ~1.7us) plus a small finisher
# reduce on the engine named second.  Pool pieces must be EARLY arrivals
# (Pool also emits SWDGE descriptors until ~12.5us).
ACCUM_PLAN = {
    "blk1": ("pool", "scalar"),
    "blk2": "scalar",
    "blk3": ("pool", "scalar"),
    "blk4": "scalar",
    "blk5": "scalar",
    "blk6": "vector",
    "blk7": "vector",
    "spill": ("pool", "vector"),
    "b0lo": "scalar",
    "b0hi": "vector",
}

# ---------------------------------------------------------------------------
# Workaround: this container's walrus encodes at most 1 sync wait per
# instruction.  Split longer on_wait lists onto preceding same-engine NoOps.
MAX_WAITS = 1
_orig_compile_bir_kernel = bass_utils.compile_bir_kernel


def _split_waits_in_bir(bir: dict) -> int:
    counter = [0]

    def fix_blocks(blocks):
        for bb in blocks:
            out = []
            for ins in bb.get("instructions", []):
                si = ins.get("sync_info")
                ow = (si or {}).get("on_wait") or []
                if len(ow) > MAX_WAITS:
                    extra, keep = ow[:-MAX_WAITS], ow[-MAX_WAITS:]
                    for i in range(0, len(extra), MAX_WAITS):
                        counter[0] += 1
                        out.append({
                            "name": f"I-waitsplit-{counter[0]}",
                            "engine": ins["engine"],
                            "opcode": "NoOp",
                            "ins": [],
                            "outs": [],
                            "debug": ins.get("debug", 0),
                            "sync_info": {
                                "on_update": [],
                                "on_wait": extra[i : i + MAX_WAITS],
                            },
                        })
                    si["on_wait"] = keep
                out.append(ins)
            bb["instructions"] = out
            if bb.get("blocks"):
                fix_blocks(bb["blocks"])

    for fn in bir["functions"]:
        fix_blocks(fn["blocks"])
    return counter[0]


def _patched_compile_bir_kernel(bir_json, tmpdir, neff_name="file.neff"):
    if isinstance(bir_json, str):
        bir_json = bir_json.encode()
    bir = json.loads(bir_json)
    _split_waits_in_bir(bir)
    return _orig_compile_bir_kernel(json.dumps(bir).encode(), tmpdir, neff_name)


bass_utils.compile_bir_kernel = _patched_compile_bir_kernel
bass2jax.compile_bir_kernel = _patched_compile_bir_kernel


# ---------------------------------------------------------------------------
# Overlapped TileContext exit.  The stock exit serializes: drain(+DMA-sem
# waits) -> all-engine barrier -> sem clears -> barrier, so every engine's
# walrus postamble starts only after the out-DMA's completion receipt.
# Instead: Sync drains with the global-clock + DMA-completion waits, then
# incs a handoff semaphore; GpSimd and Vector wait for the handoff before
# entering their postambles; Tensor and Scalar get no tail instructions at
# all.  The explicit Tile sem clears are dropped: the walrus postamble wipes
# all 256 semaphores every execution, which keeps re-execution correct.
import concourse.tile as tile_mod
from concourse.tile import TileContext as _TC


def _overlap_drain_and_barrier(self, tick_clock, wait_clock):
    nc = self.nc
    drain_inst = nc.sync.drain()
    wait_clock.add_sem_waits(
        drain_inst.ins,
        tile_mod.ScopedClock({None: tick_clock.global_clock}),
    )
    done = nc.alloc_semaphore("tail_dma_done")
    # Must not sit in Tensor's or Scalar's postamble-clear slice (they are
    # released early and would zero it while GpSimd/Vector still wait).
    assert done.num >= 105, done.num
    drain_inst.then_inc(done, 1)
    nc.gpsimd.wait_ge(done, 1)
    nc.vector.wait_ge(done, 1)
    popped = nc._tile_sem_poison_stack.pop()
    assert popped is self._sem_poison


_TC._drain_and_barrier = _overlap_drain_and_barrier
# ---------------------------------------------------------------------------


def _build():
    # Bass.__init__ ends with an all-engine barrier ordering its const-AP
    # memsets against the body.  This kernel never reads those const APs,
    # and the NRT start barrier already aligns the engines at execution
    # start, so skip it.
    _orig_aeb = bass.Bass.all_engine_barrier
    bass.Bass.all_engine_barrier = lambda self, **kw: None
    try:
        nc = bass.Bass("TRN2")
    finally:
        bass.Bass.all_engine_barrier = _orig_aeb
    x_sh = nc.dram_tensor("x_shard", (BB, KB), F32, kind="ExternalInput")
    w_sh = nc.dram_tensor("wsums_shard", (G, KB), F32, kind="ExternalInput")
    out = nc.dram_tensor("out_acc", (HOME_P, NCOL), F32, kind="ExternalOutput")
    out_sp = nc.dram_tensor("out_spill", (NSP, 1), F32, kind="ExternalOutput")

    eng = {
        "scalar": nc.scalar,
        "vector": nc.vector,
        "pool": nc.gpsimd,
    }

    pool_scratch = []

    def accumulate(name, yt, kw, acc_ap, npart):
        e = ACCUM_PLAN[name]
        if e == "scalar":
            nc.scalar.activation(
                yt, yt, mybir.ActivationFunctionType.Copy, accum_out=acc_ap
            )
        elif e == "vector":
            nc.vector.tensor_reduce(
                acc_ap, yt, axis=mybir.AxisListType.X, op=mybir.AluOpType.add
            )
        else:
            # Pool add-tree kw -> kw/8 (f32 partials: only the bf16 y terms
            # are rounded, not the partial sums), then a small finisher.
            _, fin = e
            z = pool_scratch.pop()
            w = kw // 2
            nc.gpsimd.tensor_tensor(
                z[0:npart, 0:w], yt[:, 0:w], yt[:, w:kw], op=mybir.AluOpType.add
            )
            while w > kw // 8:
                h = w // 2
                nc.gpsimd.tensor_tensor(
                    z[0:npart, 0:h],
                    z[0:npart, 0:h],
                    z[0:npart, h:w],
                    op=mybir.AluOpType.add,
                )
                w = h
            if fin == "vector":
                nc.vector.tensor_reduce(
                    acc_ap,
                    z[0:npart, 0:w],
                    axis=mybir.AxisListType.X,
                    op=mybir.AluOpType.add,
                )
            else:
                nc.scalar.activation(
                    z[0:npart, 0:w],
                    z[0:npart, 0:w],
                    mybir.ActivationFunctionType.Copy,
                    accum_out=acc_ap,
                )

    with TileContext(nc) as tc:
        with (
            tc.tile_pool(name="const", bufs=1) as cpool,
            tc.tile_pool(name="psum", bufs=1, space="PSUM") as ppool,
        ):
            # wsums slice, cast f32->bf16 in the SDMA datapath (SWDGE).
            ws = cpool.tile([G, KB], BF16)
            nc.gpsimd.dma_start(out=ws, in_=w_sh.ap())

            # Stationary = SCALE (exact in bf16): folds the output scale
            # into the broadcast matmul, so wp = SCALE * w_total.
            ones = cpool.tile([G, P], BF16)
            nc.gpsimd.memset(ones, SCALE)

            # wp_ps[m, n] = sum_g ones[g, m] * ws[g, n] = SCALE*w_total[n]
            # on every partition m.  N<=512 per matmul (one PSUM bank each).
            wp_ps = ppool.tile([P, KB], F32)
            for j in range(KB // 512):
                nc.tensor.matmul(
                    wp_ps[:, j * 512 : (j + 1) * 512],
                    ones,
                    ws[:, j * 512 : (j + 1) * 512],
                    start=True,
                    stop=True,
                )
            # PSUM f32 -> SBUF bf16 so the tensor_tensor runs in the DVE
            # 2x packed mode (PSUM operands force 1x).  ScalarE is idle
            # here and sits closer to PSUM.
            wp = cpool.tile([P, KB], BF16)
            nc.scalar.activation(wp, wp_ps, mybir.ActivationFunctionType.Copy)

            acc = cpool.tile([HOME_P, NCOL], F32)
            acc_sp = cpool.tile([NSP, 1], F32)
            n_pool_pieces = sum(
                1 for v in ACCUM_PLAN.values() if isinstance(v, tuple)
            )
            for i in range(n_pool_pieces):
                pool_scratch.append(
                    cpool.tile(
                        [HOME_P, KB // 2], F32, tag=f"pz{i}", name=f"pz{i}"
                    )
                )

            # Home pieces: (name(s), blocks, k0, kw, acc cols).  "spill"
            # marks the batched rows-124..127 piece; it is second in the
            # DMA stream (small, lands early) so its Pool accumulation
            # stays off the tail.
            pieces = [
                (("blk1", "blk2"), (1, 2), 0, KB, (0, 1)),
                ("SPILL",),
                (("blk3", "blk4"), (3, 4), 0, KB, (2, 3)),
                (("blk5", "blk6"), (5, 6), 0, KB, (4, 5)),
                (("blk7",), (7,), 0, KB, (6,)),
                (("b0lo",), (0,), 0, KB // 2, (7,)),
                (("b0hi",), (0,), KB // 2, KB // 2, (8,)),
            ]
            for pi, piece in enumerate(pieces):
                if piece == ("SPILL",):
                    # rows 124..127 of every block, full k, one partition
                    # per (block, row): partition 4*b + i <- row 128*b+124+i.
                    xs = cpool.tile([NSP, KB], BF16)
                    src = bass.AP(
                        x_sh,
                        HOME_P * KB,
                        [[P * KB, NBLK], [KB, SPILL_ROWS], [1, KB]],
                    )
                    nc.gpsimd.dma_start(out=xs, in_=src)
                    ys = cpool.tile([NSP, KB], BF16)
                    nc.vector.tensor_tensor(
                        ys, xs, wp[0:NSP, :], op=mybir.AluOpType.mult
                    )
                    accumulate("spill", ys, KB, acc_sp[:, 0:1], NSP)
                    continue
                names, blocks, k0, kw, cols = piece
                nrb = len(blocks)
                rb0 = blocks[0]
                assert blocks == tuple(range(rb0, rb0 + nrb))
                xt = cpool.tile([HOME_P, nrb * kw], BF16, tag=f"xt{pi}")
                # src[p, a, k] = x_shard[(rb0 + a) * P + p, k0 + k]
                src = bass.AP(
                    x_sh,
                    rb0 * P * KB + k0,
                    [[KB, HOME_P], [P * KB, nrb], [1, kw]],
                )
                nc.gpsimd.dma_start(out=xt, in_=src)
                yt = cpool.tile([HOME_P, nrb * kw], BF16, tag=f"yt{pi}")
                if nrb == 1:
                    nc.vector.tensor_tensor(
                        yt,
                        xt,
                        wp[0:HOME_P, k0 : k0 + kw],
                        op=mybir.AluOpType.mult,
                    )
                else:
                    # One fused multiply over nrb row-blocks; wp is repeated
                    # along a stride-0 middle dim.
                    x3 = xt[:].rearrange("p (a k) -> p a k", a=nrb)
                    y3 = yt[:].rearrange("p (a k) -> p a k", a=nrb)
                    wb = (
                        wp[0:HOME_P, k0 : k0 + kw]
                        .unsqueeze(1)
                        .broadcast_to([HOME_P, nrb, kw])
                    )
                    nc.vector.tensor_tensor(y3, x3, wb, op=mybir.AluOpType.mult)
                for a, name in enumerate(names):
                    accumulate(
                        name,
                        yt[:, a * kw : (a + 1) * kw],
                        kw,
                        acc[:, cols[a] : cols[a] + 1],
                        HOME_P,
                    )
                if pi == len(pieces) - 2:
                    # All home acc columns except the last are final; the
                    # spill partials are also done by now.  Store both
                    # while the last half-block is still in flight.
                    nc.sync.dma_start(
                        out=out.ap()[:, 0 : NCOL - 1], in_=acc[:, 0 : NCOL - 1]
                    )
                    nc.sync.dma_start(out=out_sp.ap(), in_=acc_sp)
            nc.sync.dma_start(
                out=out.ap()[:, NCOL - 1 : NCOL], in_=acc[:, NCOL - 1 : NCOL]
            )
    return nc


def kernel(x: np.ndarray, wsums: np.ndarray) -> np.ndarray:
    global _built, LAST_RESULTS
    if _built is None:
        _built = _build()
    nc = _built

    x = np.ascontiguousarray(np.asarray(x, dtype=np.float32))
    wsums = np.ascontiguousarray(np.asarray(wsums, dtype=np.float32))

    in_maps = []
    for c in range(N_CORES):
        bb_i, kb_i = divmod(c, KSHARD)
        xs = np.ascontiguousarray(
            x[bb_i * BB : (bb_i + 1) * BB, kb_i * KB : (kb_i + 1) * KB]
        )
        wsl = np.ascontiguousarray(wsums[:, kb_i * KB : (kb_i + 1) * KB])
        in_maps.append({"x_shard": xs, "wsums_shard": wsl})

    res = bass_utils.run_bass_kernel_spmd(
        nc,
        in_maps,
        core_ids=list(range(N_CORES)),
        trace=TRACE,
        **TRACE_KWARGS,
    )
    LAST_RESULTS = res

    parts = []
    for bb_i in range(BSHARD):
        tot = None
        for kb_i in range(KSHARD):
            r = res.results[bb_i * KSHARD + kb_i]
            acc = r["out_acc"]       # [HOME_P, NCOL]
            asp = r["out_spill"]     # [NSP, 1]
            blk = np.empty((NBLK, P), dtype=np.float32)
            # acc col j-1 holds block j (j=1..7); block 0 = col7 + col8.
            blk[0, :HOME_P] = acc[:, NCOL - 2] + acc[:, NCOL - 1]
            for j in range(1, NBLK):
                blk[j, :HOME_P] = acc[:, j - 1]
            # spill partition 4*b + i holds row 128*b + 124 + i.
            blk[:, HOME_P:] = asp[:, 0].reshape(NBLK, SPILL_ROWS)
            vec = blk.reshape(BB)
            tot = vec if tot is None else tot + vec
        parts.append(tot)
    return np.concatenate(parts).astype(np.float32)[:, None]


# revision 10
# speedup vs baseline: 1.4812x; 1.4812x over previous
"""Trainium2 Bass kernel for: out = (x @ wsums.sum(0)) * (1.5 * 0.5).

x: [1024, 8192] f32, wsums: [32, 8192] f32 -> out: [1024, 1] f32.

Sharding across 8 NeuronCores: 8-way along the contraction dim k
(8192 -> 1024 per core).  Each core reads a 4MB x column-shard plus its
128KB wsums k-slice, computes partial dot products for ALL 1024 rows over
its k-slice, and the host sums the 8 per-core partials (the unshard step
for a contraction-sharded dim).

v5 design (v1 f32/HWDGE: 29.4-30.2us; v2 bf16 first cut: 33.3us; v4
pool/124-partition experiment: 45.6us):
  - x and wsums are DMA'd through the SWDGE (gpsimd) path with an f32->bf16
    cast in the SDMA datapath.  HBM read bytes are unchanged (the memory
    bound) and v2 measured the same ~430GB/s aggregate read rate as HWDGE,
    but the tensor_tensor multiplies then run at the DVE 2x bf16 packed
    rate (measured 1.23us vs 2.29us per 1MB chunk).
  - All tiles are 128-partition: v4 showed that partial-partition DMAs
    (124-wide) skew the SWDGE descriptor->engine distribution badly
    (127/37/95 packets per engine vs ~74 even), collapsing DMA throughput.
  - Pool (GpSimd) is kept OFF the compute path: its tensor_tensor adds
    measured ~2.6x slower than the cost model, and any waiting compute on
    Pool's in-order stream also blocks later piece DMA triggers.
  - The per-element accumulation pass is the wall (~1 elem/cycle on DVE
    and ScalarE alike, dtype-independent: ACT 1.43us / DVE reduce 1.21us
    per 1024-wide block).  It is split arrival-aware: DVE reduces ride the
    gaps between its TTs (blk1, blk3) and take the final tail piece
    (b0hi); ScalarE takes the rest.  The wp PSUM->SBUF bf16 cast runs on
    DVE (idle before the first chunk lands) to keep ScalarE's window free.
  - Block 0 is processed last as two half-k pieces; acc columns 0..7 are
    stored while the last half-block computes, so only a 128x4B store +
    completion receipt trails the final reduce.  The host adds the two
    block-0 partials (it already sums 8 per-core partials).

Environment workarounds (this container's walrus build):
  - it encodes at most ONE semaphore wait per instruction ("Too many sync
    wait commands"), so compile_bir_kernel is wrapped with a BIR post-pass
    that moves excess waits onto preceding same-engine NoOp instructions;
  - it cannot encode bass_isa raw-ISA ops (tensor_tensor_reduce,
    partition_all_reduce, ... -> "ISA wrong length"), so only classic
    mybir ops are used.
"""

import json

import numpy as np

import concourse.bass as bass
import concourse.bass2jax as bass2jax
import concourse.bass_utils as bass_utils
import concourse.mybir as mybir
from concourse.tile import TileContext

SCALE = 1.5 * 0.5
B, K, G = 1024, 8192, 32
N_CORES = 8
KSHARD = 8                  # cores along k
BSHARD = N_CORES // KSHARD  # cores along batch
KB = K // KSHARD            # per-core k width
BB = B // BSHARD            # per-core rows
P = 128
NBLK = BB // P              # row-blocks per core
NCOL = NBLK + 1             # acc columns: blk1..7 -> 0..6, blk0 lo/hi -> 7/8
F32 = mybir.dt.float32
BF16 = mybir.dt.bfloat16

# Set by test.py to profile; results stashed in LAST_RESULTS.
TRACE = False
TRACE_KWARGS = {}
LAST_RESULTS = None

_built = None

# Accumulating engine per piece.  DVE ("vector") reduces are placed where
# DVE has arrival gaps between its TTs; ScalarE takes the rest.
ACCUM_PLAN = {
    "blk1": "vector",
    "blk2": "scalar",
    "blk3": "vector",
    "blk4": "scalar",
    "blk5": "scalar",
    "blk6": "scalar",
    "blk7": "scalar",
    "b0lo": "scalar",
    "b0hi": "vector",
}

# ---------------------------------------------------------------------------
# Workaround: this container's walrus encodes at most 1 sync wait per
# instruction.  Split longer on_wait lists onto preceding same-engine NoOps.
MAX_WAITS = 1
_orig_compile_bir_kernel = bass_utils.compile_bir_kernel


def _split_waits_in_bir(bir: dict) -> int:
    counter = [0]

    def fix_blocks(blocks):
        for bb in blocks:
            out = []
            for ins in bb.get("instructions", []):
                si = ins.get("sync_info")
                ow = (si or {}).get("on_wait") or []
                if len(ow) > MAX_WAITS:
                    extra, keep = ow[:-MAX_WAITS], ow[-MAX_WAITS:]
                    for i in range(0, len(extra), MAX_WAITS):
                        counter[0] += 1
                        out.append({
                            "name": f"I-waitsplit-{counter[0]}",
                            "engine": ins["engine"],
                            "opcode": "NoOp",
                            "ins": [],
                            "outs": [],
                            "debug": ins.get("debug", 0),
                            "sync_info": {
                                "on_update": [],
                                "on_wait": extra[i : i + MAX_WAITS],
                            },
                        })
                    si["on_wait"] = keep
                out.append(ins)
            bb["instructions"] = out
            if bb.get("blocks"):
                fix_blocks(bb["blocks"])

    for fn in bir["functions"]:
        fix_blocks(fn["blocks"])
    return counter[0]


def _patched_compile_bir_kernel(bir_json, tmpdir, neff_name="file.neff"):
    if isinstance(bir_json, str):
        bir_json = bir_json.encode()
    bir = json.loads(bir_json)
    _split_waits_in_bir(bir)
    return _orig_compile_bir_kernel(json.dumps(bir).encode(), tmpdir, neff_name)


bass_utils.compile_bir_kernel = _patched_compile_bir_kernel
bass2jax.compile_bir_kernel = _patched_compile_bir_kernel


# ---------------------------------------------------------------------------
# Overlapped TileContext exit.  The stock exit serializes: drain(+DMA-sem
# waits) -> all-engine barrier -> sem clears -> barrier, so every engine's
# walrus postamble starts only after the out-DMA's completion receipt.
# Instead: Sync drains with the global-clock + DMA-completion waits, then
# incs a handoff semaphore; GpSimd and Vector wait for the handoff before
# entering their postambles; Tensor and Scalar get no tail instructions at
# all.  The explicit Tile sem clears are dropped: the walrus postamble wipes
# all 256 semaphores every execution, which keeps re-execution correct.
import concourse.tile as tile_mod
from concourse.tile import TileContext as _TC


def _overlap_drain_and_barrier(self, tick_clock, wait_clock):
    nc = self.nc
    drain_inst = nc.sync.drain()
    wait_clock.add_sem_waits(
        drain_inst.ins,
        tile_mod.ScopedClock({None: tick_clock.global_clock}),
    )
    done = nc.alloc_semaphore("tail_dma_done")
    # Must not sit in Tensor's or Scalar's postamble-clear slice (they are
    # released early and would zero it while GpSimd/Vector still wait).
    assert done.num >= 105, done.num
    drain_inst.then_inc(done, 1)
    nc.gpsimd.wait_ge(done, 1)
    nc.vector.wait_ge(done, 1)
    popped = nc._tile_sem_poison_stack.pop()
    assert popped is self._sem_poison


_TC._drain_and_barrier = _overlap_drain_and_barrier
# ---------------------------------------------------------------------------


def _build():
    # Bass.__init__ ends with an all-engine barrier ordering its const-AP
    # memsets against the body.  This kernel never reads those const APs,
    # and the NRT start barrier already aligns the engines at execution
    # start, so skip it.
    _orig_aeb = bass.Bass.all_engine_barrier
    bass.Bass.all_engine_barrier = lambda self, **kw: None
    try:
        nc = bass.Bass("TRN2")
    finally:
        bass.Bass.all_engine_barrier = _orig_aeb
    x_sh = nc.dram_tensor("x_shard", (BB, KB), F32, kind="ExternalInput")
    w_sh = nc.dram_tensor("wsums_shard", (G, KB), F32, kind="ExternalInput")
    out = nc.dram_tensor("out_acc", (P, NCOL), F32, kind="ExternalOutput")

    def accumulate(name, yt, acc_ap):
        if ACCUM_PLAN[name] == "scalar":
            nc.scalar.activation(
                yt, yt, mybir.ActivationFunctionType.Copy, accum_out=acc_ap
            )
        else:
            nc.vector.tensor_reduce(
                acc_ap, yt, axis=mybir.AxisListType.X, op=mybir.AluOpType.add
            )

    with TileContext(nc) as tc:
        with (
            tc.tile_pool(name="const", bufs=1) as cpool,
            tc.tile_pool(name="psum", bufs=1, space="PSUM") as ppool,
        ):
            # wsums slice, cast f32->bf16 in the SDMA datapath (SWDGE).
            ws = cpool.tile([G, KB], BF16)
            nc.gpsimd.dma_start(out=ws, in_=w_sh.ap())

            # Pieces in DMA/processing order: (names, blocks, k0, kw, cols).
            pieces = [
                (("blk1", "blk2"), (1, 2), 0, KB, (0, 1)),
                (("blk3", "blk4"), (3, 4), 0, KB, (2, 3)),
                (("blk5", "blk6"), (5, 6), 0, KB, (4, 5)),
                (("blk7",), (7,), 0, KB, (6,)),
                (("b0lo",), (0,), 0, KB // 2, (7,)),
                (("b0hi",), (0,), KB // 2, KB // 2, (8,)),
            ]

            # Trigger every piece's DMA up-front on GpSimd so the SWDGE
            # emission (~0.7-1.4us per piece, serial on Q7) finishes as
            # early as possible; all x tiles are SBUF-resident.
            xts = []
            for pi, (names, blocks, k0, kw, cols) in enumerate(pieces):
                nrb = len(blocks)
                rb0 = blocks[0]
                assert blocks == tuple(range(rb0, rb0 + nrb))
                xt = cpool.tile(
                    [P, nrb * kw], BF16, tag=f"xt{pi}", name=f"xt{pi}"
                )
                # src[p, a, k] = x_shard[(rb0 + a) * P + p, k0 + k]
                src = bass.AP(
                    x_sh,
                    rb0 * P * KB + k0,
                    [[KB, P], [P * KB, nrb], [1, kw]],
                )
                nc.gpsimd.dma_start(out=xt, in_=src)
                xts.append(xt)
                if pi == 0:
                    # Stationary = SCALE (exact in bf16): folds the output
                    # scale into the broadcast matmul.  Emitted after the
                    # first big x DMA trigger; the matmul needs it ~2.5us
                    # later than that.
                    ones = cpool.tile([G, P], BF16, name="ones")
                    nc.gpsimd.memset(ones, SCALE)

            # wp_ps[m, n] = sum_g ones[g, m] * ws[g, n] = SCALE*w_total[n]
            # on every partition m.  N<=512 per matmul (one PSUM bank each).
            wp_ps = ppool.tile([P, KB], F32)
            for j in range(KB // 512):
                nc.tensor.matmul(
                    wp_ps[:, j * 512 : (j + 1) * 512],
                    ones,
                    ws[:, j * 512 : (j + 1) * 512],
                    start=True,
                    stop=True,
                )
            # PSUM f32 -> SBUF bf16 so the tensor_tensor runs in the DVE
            # 2x packed mode (PSUM operands force 1x).  On DVE: it is idle
            # until the first chunk lands, and this keeps ScalarE's
            # accumulation window clear.
            wp = cpool.tile([P, KB], BF16)
            nc.vector.tensor_copy(wp, wp_ps)

            acc = cpool.tile([P, NCOL], F32)

            for pi, (names, blocks, k0, kw, cols) in enumerate(pieces):
                nrb = len(blocks)
                xt = xts[pi]
                yt = cpool.tile(
                    [P, nrb * kw], BF16, tag=f"yt{pi}", name=f"yt{pi}"
                )
                if nrb == 1:
                    nc.vector.tensor_tensor(
                        yt, xt, wp[:, k0 : k0 + kw], op=mybir.AluOpType.mult
                    )
                else:
                    # One fused multiply over nrb row-blocks; wp is repeated
                    # along a stride-0 middle dim.
                    x3 = xt[:].rearrange("p (a k) -> p a k", a=nrb)
                    y3 = yt[:].rearrange("p (a k) -> p a k", a=nrb)
                    wb = (
                        wp[:, k0 : k0 + kw]
                        .unsqueeze(1)
                        .broadcast_to([P, nrb, kw])
                    )
                    nc.vector.tensor_tensor(y3, x3, wb, op=mybir.AluOpType.mult)
                for a, name in enumerate(names):
                    accumulate(
                        name,
                        yt[:, a * kw : (a + 1) * kw],
                        acc[:, cols[a] : cols[a] + 1],
                    )
                if pi == len(pieces) - 2:
                    # All acc columns except the last are final: store them
                    # while the last half-block is still in flight.
                    nc.sync.dma_start(
                        out=out.ap()[:, 0 : NCOL - 1], in_=acc[:, 0 : NCOL - 1]
                    )
            nc.sync.dma_start(
                out=out.ap()[:, NCOL - 1 : NCOL], in_=acc[:, NCOL - 1 : NCOL]
            )
    return nc


def kernel(x: np.ndarray, wsums: np.ndarray) -> np.ndarray:
    global _built, LAST_RESULTS
    if _built is None:
        _built = _build()
    nc = _built

    x = np.ascontiguousarray(np.asarray(x, dtype=np.float32))
    wsums = np.ascontiguousarray(np.asarray(wsums, dtype=np.float32))

    in_maps = []
    for c in range(N_CORES):
        bb_i, kb_i = divmod(c, KSHARD)
        xs = np.ascontiguousarray(
            x[bb_i * BB : (bb_i + 1) * BB, kb_i * KB : (kb_i + 1) * KB]
        )
        wsl = np.ascontiguousarray(wsums[:, kb_i * KB : (kb_i + 1) * KB])
        in_maps.append({"x_shard": xs, "wsums_shard": wsl})

    res = bass_utils.run_bass_kernel_spmd(
        nc,
        in_maps,
        core_ids=list(range(N_CORES)),
        trace=TRACE,
        **TRACE_KWARGS,
    )
    LAST_RESULTS = res

    parts = []
    for bb_i in range(BSHARD):
        tot = None
        for kb_i in range(KSHARD):
            acc = res.results[bb_i * KSHARD + kb_i]["out_acc"]  # [P, NCOL]
            # acc col j-1 holds block j (j=1..7); block 0 = col7 + col8.
            blk = np.concatenate(
                [
                    (acc[:, NCOL - 2] + acc[:, NCOL - 1])[:, None],
                    acc[:, 0 : NBLK - 1],
                ],
                axis=1,
            )  # [P, NBLK] in block order 0..7
            vec = blk.T.reshape(BB)  # row 128*j + p  <-  blk[p, j]
            tot = vec if tot is None else tot + vec
        parts.append(tot)
    return np.concatenate(parts).astype(np.float32)[:, None]


# revision 19
# speedup vs baseline: 1.5285x; 1.0319x over previous
"""Trainium2 Bass kernel for: out = (x @ wsums.sum(0)) * (1.5 * 0.5).

x: [1024, 8192] f32, wsums: [32, 8192] f32 -> out: [1024, 1] f32.

Sharding across 8 NeuronCores: 8-way along the contraction dim k
(8192 -> 1024 per core).  Each core reads a 4MB x column-shard plus its
128KB wsums k-slice, computes partial dot products for ALL 1024 rows over
its k-slice, and the host sums the 8 per-core partials (the unshard step
for a contraction-sharded dim).

v5 design (v1 f32/HWDGE: 29.4-30.2us; v2 bf16 first cut: 33.3us; v4
pool/124-partition experiment: 45.6us):
  - x and wsums are DMA'd through the SWDGE (gpsimd) path with an f32->bf16
    cast in the SDMA datapath.  HBM read bytes are unchanged (the memory
    bound) and v2 measured the same ~430GB/s aggregate read rate as HWDGE,
    but the tensor_tensor multiplies then run at the DVE 2x bf16 packed
    rate (measured 1.23us vs 2.29us per 1MB chunk).
  - All tiles are 128-partition: v4 showed that partial-partition DMAs
    (124-wide) skew the SWDGE descriptor->engine distribution badly
    (127/37/95 packets per engine vs ~74 even), collapsing DMA throughput.
  - Pool (GpSimd) is kept OFF the compute path: its tensor_tensor adds
    measured ~2.6x slower than the cost model, and any waiting compute on
    Pool's in-order stream also blocks later piece DMA triggers.
  - The per-element accumulation pass is the wall (~1 elem/cycle on DVE
    and ScalarE alike, dtype-independent: ACT 1.43us / DVE reduce 1.21us
    per 1024-wide block).  It is split arrival-aware: DVE reduces ride the
    gaps between its TTs (blk1, blk3) and take the final tail piece
    (b0hi); ScalarE takes the rest.  The wp PSUM->SBUF bf16 cast runs on
    DVE (idle before the first chunk lands) to keep ScalarE's window free.
  - Block 0 is processed last as two half-k pieces; acc columns 0..7 are
    stored while the last half-block computes, so only a 128x4B store +
    completion receipt trails the final reduce.  The host adds the two
    block-0 partials (it already sums 8 per-core partials).

Environment workarounds (this container's walrus build):
  - it encodes at most ONE semaphore wait per instruction ("Too many sync
    wait commands"), so compile_bir_kernel is wrapped with a BIR post-pass
    that moves excess waits onto preceding same-engine NoOp instructions;
  - it cannot encode bass_isa raw-ISA ops (tensor_tensor_reduce,
    partition_all_reduce, ... -> "ISA wrong length"), so only classic
    mybir ops are used.
"""

import json

import numpy as np

import concourse.bass as bass
import concourse.bass2jax as bass2jax
import concourse.bass_utils as bass_utils
import concourse.mybir as mybir
from concourse.tile import TileContext

SCALE = 1.5 * 0.5
B, K, G = 1024, 8192, 32
N_CORES = 8
KSHARD = 8                  # cores along k
BSHARD = N_CORES // KSHARD  # cores along batch
KB = K // KSHARD            # per-core k width
BB = B // BSHARD            # per-core rows
P = 128
NBLK = BB // P              # row-blocks per core
# acc columns: blk1..6 -> 0..5, blk7 k-lo/hi -> 6/7, blk0 k-lo/hi -> 8/9
NCOL = NBLK + 2
F32 = mybir.dt.float32
BF16 = mybir.dt.bfloat16

# Set by test.py to profile; results stashed in LAST_RESULTS.
TRACE = False
TRACE_KWARGS = {}
LAST_RESULTS = None

_built = None

# Accumulating engine per piece.  DVE ("vector") reduces are placed where
# DVE has arrival gaps between its TTs; ScalarE takes the rest.  The late
# pieces (blk7, blk0) are k-halves so their accumulations spread across
# both engines at the tail.
ACCUM_PLAN = {
    "blk1": "vector",
    "blk2": "scalar",
    "blk3": "vector",
    "blk4": "scalar",
    "blk5": "scalar",
    "blk6": "scalar",
    "b7lo": "scalar",
    "b7hi": "vector",
    "b0lo": "scalar",
    "b0hi": "vector",
}

# ---------------------------------------------------------------------------
# Workaround: this container's walrus encodes at most 1 sync wait per
# instruction.  Split longer on_wait lists onto preceding same-engine NoOps.
MAX_WAITS = 1
_orig_compile_bir_kernel = bass_utils.compile_bir_kernel


def _split_waits_in_bir(bir: dict) -> int:
    counter = [0]

    def fix_blocks(blocks):
        for bb in blocks:
            out = []
            for ins in bb.get("instructions", []):
                si = ins.get("sync_info")
                ow = (si or {}).get("on_wait") or []
                if len(ow) > MAX_WAITS:
                    extra, keep = ow[:-MAX_WAITS], ow[-MAX_WAITS:]
                    for i in range(0, len(extra), MAX_WAITS):
                        counter[0] += 1
                        out.append({
                            "name": f"I-waitsplit-{counter[0]}",
                            "engine": ins["engine"],
                            "opcode": "NoOp",
                            "ins": [],
                            "outs": [],
                            "debug": ins.get("debug", 0),
                            "sync_info": {
                                "on_update": [],
                                "on_wait": extra[i : i + MAX_WAITS],
                            },
                        })
                    si["on_wait"] = keep
                out.append(ins)
            bb["instructions"] = out
            if bb.get("blocks"):
                fix_blocks(bb["blocks"])

    for fn in bir["functions"]:
        fix_blocks(fn["blocks"])
    return counter[0]


def _patched_compile_bir_kernel(bir_json, tmpdir, neff_name="file.neff"):
    if isinstance(bir_json, str):
        bir_json = bir_json.encode()
    bir = json.loads(bir_json)
    _split_waits_in_bir(bir)
    return _orig_compile_bir_kernel(json.dumps(bir).encode(), tmpdir, neff_name)


bass_utils.compile_bir_kernel = _patched_compile_bir_kernel
bass2jax.compile_bir_kernel = _patched_compile_bir_kernel

# ---------------------------------------------------------------------------
# The walrus postamble zeroes all 256 semaphores (51 per engine) after the
# body, ~6.7us of the measured window on every execution.  The kernel only
# allocates ~25 sems, so cap the semaphore space: the wipe shrinks
# proportionally and still covers every allocatable sem, keeping
# re-execution correct by construction.
MAX_SEM_NUM = None  # 64 made the kernel produce NaNs (sem renumbering race)
_orig_get_walrus_args = bass_utils.get_walrus_args


def _patched_get_walrus_args(*a, **k):
    extra = [] if MAX_SEM_NUM is None else [f"--max-sem-num={MAX_SEM_NUM}"]
    return [*_orig_get_walrus_args(*a, **k), *extra]


bass_utils.get_walrus_args = _patched_get_walrus_args


# ---------------------------------------------------------------------------
# Overlapped TileContext exit.  The stock exit serializes: drain(+DMA-sem
# waits) -> all-engine barrier -> sem clears -> barrier, so every engine's
# walrus postamble starts only after the out-DMA's completion receipt.
# Instead: Sync drains with the global-clock + DMA-completion waits, then
# incs a handoff semaphore; GpSimd and Vector wait for the handoff before
# entering their postambles; Tensor and Scalar get no tail instructions at
# all.  The explicit Tile sem clears are dropped: the walrus postamble wipes
# all 256 semaphores every execution, which keeps re-execution correct.
import concourse.tile as tile_mod
from concourse.tile import TileContext as _TC


def _overlap_drain_and_barrier(self, tick_clock, wait_clock):
    nc = self.nc
    drain_inst = nc.sync.drain()
    wait_clock.add_sem_waits(
        drain_inst.ins,
        tile_mod.ScopedClock({None: tick_clock.global_clock}),
    )
    done = nc.alloc_semaphore("tail_dma_done")
    # Tensor/Scalar's early postambles may zero this sem, but they run long
    # before Sync's drain increments it, so a clear-then-inc is harmless.
    drain_inst.then_inc(done, 1)
    nc.gpsimd.wait_ge(done, 1)
    nc.vector.wait_ge(done, 1)
    popped = nc._tile_sem_poison_stack.pop()
    assert popped is self._sem_poison


_TC._drain_and_barrier = _overlap_drain_and_barrier
# ---------------------------------------------------------------------------


def _build():
    # Bass.__init__ ends with an all-engine barrier ordering its const-AP
    # memsets against the body.  This kernel never reads those const APs,
    # and the NRT start barrier already aligns the engines at execution
    # start, so skip it.
    _orig_aeb = bass.Bass.all_engine_barrier
    bass.Bass.all_engine_barrier = lambda self, **kw: None
    try:
        nc = bass.Bass("TRN2")
    finally:
        bass.Bass.all_engine_barrier = _orig_aeb
    x_sh = nc.dram_tensor("x_shard", (BB, KB), F32, kind="ExternalInput")
    w_sh = nc.dram_tensor("wsums_shard", (G, KB), F32, kind="ExternalInput")
    out = nc.dram_tensor("out_acc", (P, NCOL), F32, kind="ExternalOutput")

    def accumulate(name, yt, acc_ap):
        if ACCUM_PLAN[name] == "scalar":
            nc.scalar.activation(
                yt, yt, mybir.ActivationFunctionType.Copy, accum_out=acc_ap
            )
        else:
            nc.vector.tensor_reduce(
                acc_ap, yt, axis=mybir.AxisListType.X, op=mybir.AluOpType.add
            )

    with TileContext(nc) as tc:
        with (
            tc.tile_pool(name="const", bufs=1) as cpool,
            tc.tile_pool(name="psum", bufs=1, space="PSUM") as ppool,
        ):
            # wsums slice, cast f32->bf16 in the SDMA datapath (SWDGE).
            ws = cpool.tile([G, KB], BF16)
            nc.gpsimd.dma_start(out=ws, in_=w_sh.ap())

            # Pieces in DMA/processing order: (names, blocks, k0, kw, cols).
            # A piece's accumulation splits evenly over its names (for the
            # 1-block blk7 piece that means two k-halves).
            pieces = [
                (("blk1", "blk2"), (1, 2), 0, KB, (0, 1)),
                (("blk3", "blk4"), (3, 4), 0, KB, (2, 3)),
                (("blk5", "blk6"), (5, 6), 0, KB, (4, 5)),
                (("b7lo", "b7hi"), (7,), 0, KB, (6, 7)),
                (("b0lo",), (0,), 0, KB // 2, (8,)),
                (("b0hi",), (0,), KB // 2, KB // 2, (9,)),
            ]

            # Trigger every piece's DMA up-front on GpSimd so the SWDGE
            # emission (~0.7-1.4us per piece, serial on Q7) finishes as
            # early as possible; all x tiles are SBUF-resident.
            xts = []
            for pi, (names, blocks, k0, kw, cols) in enumerate(pieces):
                nrb = len(blocks)
                rb0 = blocks[0]
                assert blocks == tuple(range(rb0, rb0 + nrb))
                xt = cpool.tile(
                    [P, nrb * kw], BF16, tag=f"xt{pi}", name=f"xt{pi}"
                )
                # src[p, a, k] = x_shard[(rb0 + a) * P + p, k0 + k]
                src = bass.AP(
                    x_sh,
                    rb0 * P * KB + k0,
                    [[KB, P], [P * KB, nrb], [1, kw]],
                )
                nc.gpsimd.dma_start(out=xt, in_=src)
                xts.append(xt)
                if pi == 0:
                    # Stationary = SCALE (exact in bf16): folds the output
                    # scale into the broadcast matmul.  Emitted after the
                    # first big x DMA trigger; the matmul needs it ~2.5us
                    # later than that.
                    ones = cpool.tile([G, P], BF16, name="ones")
                    nc.gpsimd.memset(ones, SCALE)

            # wp_ps[m, n] = sum_g ones[g, m] * ws[g, n] = SCALE*w_total[n]
            # on every partition m.  N<=512 per matmul (one PSUM bank each).
            wp_ps = ppool.tile([P, KB], F32)
            for j in range(KB // 512):
                nc.tensor.matmul(
                    wp_ps[:, j * 512 : (j + 1) * 512],
                    ones,
                    ws[:, j * 512 : (j + 1) * 512],
                    start=True,
                    stop=True,
                )
            # PSUM f32 -> SBUF bf16 so the tensor_tensor runs in the DVE
            # 2x packed mode (PSUM operands force 1x).  On ScalarE: it is
            # idle until the first accumulation (~4us later) and sits
            # closer to PSUM, while DVE's budget is the tighter one.
            wp = cpool.tile([P, KB], BF16)
            nc.scalar.activation(wp, wp_ps, mybir.ActivationFunctionType.Copy)

            acc = cpool.tile([P, NCOL], F32)

            for pi, (names, blocks, k0, kw, cols) in enumerate(pieces):
                nrb = len(blocks)
                xt = xts[pi]
                yt = cpool.tile(
                    [P, nrb * kw], BF16, tag=f"yt{pi}", name=f"yt{pi}"
                )
                if nrb == 1:
                    nc.vector.tensor_tensor(
                        yt, xt, wp[:, k0 : k0 + kw], op=mybir.AluOpType.mult
                    )
                else:
                    # One fused multiply over nrb row-blocks; wp is repeated
                    # along a stride-0 middle dim.
                    x3 = xt[:].rearrange("p (a k) -> p a k", a=nrb)
                    y3 = yt[:].rearrange("p (a k) -> p a k", a=nrb)
                    wb = (
                        wp[:, k0 : k0 + kw]
                        .unsqueeze(1)
                        .broadcast_to([P, nrb, kw])
                    )
                    nc.vector.tensor_tensor(y3, x3, wb, op=mybir.AluOpType.mult)
                seg = (nrb * kw) // len(names)
                for a, name in enumerate(names):
                    accumulate(
                        name,
                        yt[:, a * seg : (a + 1) * seg],
                        acc[:, cols[a] : cols[a] + 1],
                    )
                if pi == 3:
                    # Columns 0..7 (blk1-7) are final once the blk7 halves
                    # are accumulated: store them while block 0 is still
                    # in flight; only the 2-column block-0 store plus its
                    # completion receipt trail the final reduce.
                    nc.sync.dma_start(
                        out=out.ap()[:, 0 : NCOL - 2], in_=acc[:, 0 : NCOL - 2]
                    )
            nc.sync.dma_start(
                out=out.ap()[:, NCOL - 2 : NCOL], in_=acc[:, NCOL - 2 : NCOL]
            )
    return nc


def kernel(x: np.ndarray, wsums: np.ndarray) -> np.ndarray:
    global _built, LAST_RESULTS
    if _built is None:
        _built = _build()
    nc = _built

    x = np.ascontiguousarray(np.asarray(x, dtype=np.float32))
    wsums = np.ascontiguousarray(np.asarray(wsums, dtype=np.float32))

    in_maps = []
    for c in range(N_CORES):
        bb_i, kb_i = divmod(c, KSHARD)
        xs = np.ascontiguousarray(
            x[bb_i * BB : (bb_i + 1) * BB, kb_i * KB : (kb_i + 1) * KB]
        )
        wsl = np.ascontiguousarray(wsums[:, kb_i * KB : (kb_i + 1) * KB])
        in_maps.append({"x_shard": xs, "wsums_shard": wsl})

    res = bass_utils.run_bass_kernel_spmd(
        nc,
        in_maps,
        core_ids=list(range(N_CORES)),
        trace=TRACE,
        **TRACE_KWARGS,
    )
    LAST_RESULTS = res

    parts = []
    for bb_i in range(BSHARD):
        tot = None
        for kb_i in range(KSHARD):
            acc = res.results[bb_i * KSHARD + kb_i]["out_acc"]  # [P, NCOL]
            # cols 0..5 = blk1..6; blk7 = col6+col7; blk0 = col8+col9.
            blk = np.concatenate(
                [
                    (acc[:, 8] + acc[:, 9])[:, None],
                    acc[:, 0:6],
                    (acc[:, 6] + acc[:, 7])[:, None],
                ],
                axis=1,
            )  # [P, NBLK] in block order 0..7
            vec = blk.T.reshape(BB)  # row 128*j + p  <-  blk[p, j]
            tot = vec if tot is None else tot + vec
        parts.append(tot)
    return np.concatenate(parts).astype(np.float32)[:, None]


# revision 21
# speedup vs baseline: 1.5792x; 1.0332x over previous
"""Trainium2 Bass kernel for: out = (x @ wsums.sum(0)) * (1.5 * 0.5).

x: [1024, 8192] f32, wsums: [32, 8192] f32 -> out: [1024, 1] f32.

Sharding across 8 NeuronCores: 8-way along the contraction dim k
(8192 -> 1024 per core).  Each core reads a 4MB x column-shard plus its
128KB wsums k-slice, computes partial dot products for ALL 1024 rows over
its k-slice, and the host sums the 8 per-core partials (the unshard step
for a contraction-sharded dim).

v5 design (v1 f32/HWDGE: 29.4-30.2us; v2 bf16 first cut: 33.3us; v4
pool/124-partition experiment: 45.6us):
  - x and wsums are DMA'd through the SWDGE (gpsimd) path with an f32->bf16
    cast in the SDMA datapath.  HBM read bytes are unchanged (the memory
    bound) and v2 measured the same ~430GB/s aggregate read rate as HWDGE,
    but the tensor_tensor multiplies then run at the DVE 2x bf16 packed
    rate (measured 1.23us vs 2.29us per 1MB chunk).
  - All tiles are 128-partition: v4 showed that partial-partition DMAs
    (124-wide) skew the SWDGE descriptor->engine distribution badly
    (127/37/95 packets per engine vs ~74 even), collapsing DMA throughput.
  - Pool (GpSimd) is kept OFF the compute path: its tensor_tensor adds
    measured ~2.6x slower than the cost model, and any waiting compute on
    Pool's in-order stream also blocks later piece DMA triggers.
  - The per-element accumulation pass is the wall (~1 elem/cycle on DVE
    and ScalarE alike, dtype-independent: ACT 1.43us / DVE reduce 1.21us
    per 1024-wide block).  It is split arrival-aware: DVE reduces ride the
    gaps between its TTs (blk1, blk3) and take the final tail piece
    (b0hi); ScalarE takes the rest.  The wp PSUM->SBUF bf16 cast runs on
    DVE (idle before the first chunk lands) to keep ScalarE's window free.
  - Block 0 is processed last as two half-k pieces; acc columns 0..7 are
    stored while the last half-block computes, so only a 128x4B store +
    completion receipt trails the final reduce.  The host adds the two
    block-0 partials (it already sums 8 per-core partials).

Environment workarounds (this container's walrus build):
  - it encodes at most ONE semaphore wait per instruction ("Too many sync
    wait commands"), so compile_bir_kernel is wrapped with a BIR post-pass
    that moves excess waits onto preceding same-engine NoOp instructions;
  - it cannot encode bass_isa raw-ISA ops (tensor_tensor_reduce,
    partition_all_reduce, ... -> "ISA wrong length"), so only classic
    mybir ops are used.
"""

import json

import numpy as np

import concourse.bass as bass
import concourse.bass2jax as bass2jax
import concourse.bass_utils as bass_utils
import concourse.mybir as mybir
from concourse.tile import TileContext

SCALE = 1.5 * 0.5
B, K, G = 1024, 8192, 32
N_CORES = 8
KSHARD = 8                  # cores along k
BSHARD = N_CORES // KSHARD  # cores along batch
KB = K // KSHARD            # per-core k width
BB = B // BSHARD            # per-core rows
P = 128
NBLK = BB // P              # row-blocks per core
# acc columns: blk1..6 -> 0..5, blk7 k-lo/hi -> 6/7, blk0 k-lo/hi -> 8/9
NCOL = NBLK + 2
F32 = mybir.dt.float32
BF16 = mybir.dt.bfloat16

# Set by test.py to profile; results stashed in LAST_RESULTS.
TRACE = False
TRACE_KWARGS = {}
LAST_RESULTS = None

_built = None

# Accumulating engine per piece.  DVE ("vector") reduces are placed where
# DVE has arrival gaps between its TTs; ScalarE takes the rest.  The late
# pieces (blk7, blk0) are k-halves so their accumulations spread across
# both engines at the tail.
ACCUM_PLAN = {
    "blk1": "vector",
    "blk2": "scalar",
    "blk3": "vector",
    "blk4": "scalar",
    "blk5": "scalar",
    "blk6": "scalar",
    "b7lo": "scalar",
    "b7hi": "vector",
    "b0lo": "scalar",
    "b0hi": "vector",
}

# ---------------------------------------------------------------------------
# Workaround: this container's walrus encodes at most 1 sync wait per
# instruction.  Split longer on_wait lists onto preceding same-engine NoOps.
MAX_WAITS = 1
_orig_compile_bir_kernel = bass_utils.compile_bir_kernel


def _split_waits_in_bir(bir: dict) -> int:
    counter = [0]

    def fix_blocks(blocks):
        for bb in blocks:
            out = []
            for ins in bb.get("instructions", []):
                si = ins.get("sync_info")
                ow = (si or {}).get("on_wait") or []
                if len(ow) > MAX_WAITS:
                    extra, keep = ow[:-MAX_WAITS], ow[-MAX_WAITS:]
                    for i in range(0, len(extra), MAX_WAITS):
                        counter[0] += 1
                        out.append({
                            "name": f"I-waitsplit-{counter[0]}",
                            "engine": ins["engine"],
                            "opcode": "NoOp",
                            "ins": [],
                            "outs": [],
                            "debug": ins.get("debug", 0),
                            "sync_info": {
                                "on_update": [],
                                "on_wait": extra[i : i + MAX_WAITS],
                            },
                        })
                    si["on_wait"] = keep
                out.append(ins)
            bb["instructions"] = out
            if bb.get("blocks"):
                fix_blocks(bb["blocks"])

    for fn in bir["functions"]:
        fix_blocks(fn["blocks"])
    return counter[0]


def _patched_compile_bir_kernel(bir_json, tmpdir, neff_name="file.neff"):
    if isinstance(bir_json, str):
        bir_json = bir_json.encode()
    bir = json.loads(bir_json)
    _split_waits_in_bir(bir)
    return _orig_compile_bir_kernel(json.dumps(bir).encode(), tmpdir, neff_name)


bass_utils.compile_bir_kernel = _patched_compile_bir_kernel
bass2jax.compile_bir_kernel = _patched_compile_bir_kernel

# ---------------------------------------------------------------------------
# The walrus postamble zeroes all 256 semaphores (51 per engine) after the
# body, ~6.7us of the measured window on every execution.  The kernel only
# allocates ~25 sems, so cap the semaphore space: the wipe shrinks
# proportionally and still covers every allocatable sem, keeping
# re-execution correct by construction.
MAX_SEM_NUM = None  # 64 made the kernel produce NaNs (sem renumbering race)
_orig_get_walrus_args = bass_utils.get_walrus_args


def _patched_get_walrus_args(*a, **k):
    extra = [] if MAX_SEM_NUM is None else [f"--max-sem-num={MAX_SEM_NUM}"]
    return [*_orig_get_walrus_args(*a, **k), *extra]


bass_utils.get_walrus_args = _patched_get_walrus_args


# ---------------------------------------------------------------------------
# Overlapped TileContext exit.  The stock exit serializes: drain(+DMA-sem
# waits) -> all-engine barrier -> sem clears -> barrier, so every engine's
# walrus postamble starts only after the out-DMA's completion receipt.
# Instead: Sync drains with the global-clock + DMA-completion waits, then
# incs a handoff semaphore; GpSimd and Vector wait for the handoff before
# entering their postambles; Tensor and Scalar get no tail instructions at
# all.  The explicit Tile sem clears are dropped: the walrus postamble wipes
# all 256 semaphores every execution, which keeps re-execution correct.
import concourse.tile as tile_mod
from concourse.tile import TileContext as _TC


def _overlap_drain_and_barrier(self, tick_clock, wait_clock):
    nc = self.nc
    drain_inst = nc.sync.drain()
    wait_clock.add_sem_waits(
        drain_inst.ins,
        tile_mod.ScopedClock({None: tick_clock.global_clock}),
    )
    done = nc.alloc_semaphore("tail_dma_done")
    # Tensor/Scalar's early postambles may zero this sem, but they run long
    # before Sync's drain increments it, so a clear-then-inc is harmless.
    drain_inst.then_inc(done, 1)
    nc.gpsimd.wait_ge(done, 1)
    nc.vector.wait_ge(done, 1)
    popped = nc._tile_sem_poison_stack.pop()
    assert popped is self._sem_poison


_TC._drain_and_barrier = _overlap_drain_and_barrier
# ---------------------------------------------------------------------------


def _build():
    # Bass.__init__ ends with an all-engine barrier ordering its const-AP
    # memsets against the body.  This kernel never reads those const APs,
    # and the NRT start barrier already aligns the engines at execution
    # start, so skip it.
    _orig_aeb = bass.Bass.all_engine_barrier
    bass.Bass.all_engine_barrier = lambda self, **kw: None
    try:
        nc = bass.Bass("TRN2")
    finally:
        bass.Bass.all_engine_barrier = _orig_aeb
    x_sh = nc.dram_tensor("x_shard", (BB, KB), F32, kind="ExternalInput")
    w_sh = nc.dram_tensor("wsums_shard", (G, KB), F32, kind="ExternalInput")
    out = nc.dram_tensor("out_acc", (P, NCOL), F32, kind="ExternalOutput")

    def accumulate(name, yt, acc_ap):
        if ACCUM_PLAN[name] == "scalar":
            nc.scalar.activation(
                yt, yt, mybir.ActivationFunctionType.Copy, accum_out=acc_ap
            )
        else:
            nc.vector.tensor_reduce(
                acc_ap, yt, axis=mybir.AxisListType.X, op=mybir.AluOpType.add
            )

    with TileContext(nc) as tc:
        with (
            tc.tile_pool(name="const", bufs=1) as cpool,
            tc.tile_pool(name="psum", bufs=1, space="PSUM") as ppool,
        ):
            # wsums slice, cast f32->bf16 in the SDMA datapath (SWDGE).
            ws = cpool.tile([G, KB], BF16)
            nc.gpsimd.dma_start(out=ws, in_=w_sh.ap())

            # Pieces in DMA/processing order: (names, blocks, k0, kw, cols).
            # Single-block DMAs: each completion semaphore needs only 8
            # descriptors from the slow SDMA engine (15) instead of 16, so
            # arrivals run ~1.3us earlier on straggler cores.  A piece's
            # accumulation splits evenly over its names (for blk7 that
            # means two k-halves, one per engine, at the tail).
            pieces = [
                (("blk1",), (1,), 0, KB, (0,)),
                (("blk2",), (2,), 0, KB, (1,)),
                (("blk3",), (3,), 0, KB, (2,)),
                (("blk4",), (4,), 0, KB, (3,)),
                (("blk5",), (5,), 0, KB, (4,)),
                (("blk6",), (6,), 0, KB, (5,)),
                (("b7lo", "b7hi"), (7,), 0, KB, (6, 7)),
                (("b0lo",), (0,), 0, KB // 2, (8,)),
                (("b0hi",), (0,), KB // 2, KB // 2, (9,)),
            ]

            # Trigger every piece's DMA up-front on GpSimd so the SWDGE
            # emission (~0.7-1.4us per piece, serial on Q7) finishes as
            # early as possible; all x tiles are SBUF-resident.
            xts = []
            for pi, (names, blocks, k0, kw, cols) in enumerate(pieces):
                nrb = len(blocks)
                rb0 = blocks[0]
                assert blocks == tuple(range(rb0, rb0 + nrb))
                xt = cpool.tile(
                    [P, nrb * kw], BF16, tag=f"xt{pi}", name=f"xt{pi}"
                )
                # src[p, a, k] = x_shard[(rb0 + a) * P + p, k0 + k]
                src = bass.AP(
                    x_sh,
                    rb0 * P * KB + k0,
                    [[KB, P], [P * KB, nrb], [1, kw]],
                )
                nc.gpsimd.dma_start(out=xt, in_=src)
                xts.append(xt)
                if pi == 0:
                    # Stationary = SCALE (exact in bf16): folds the output
                    # scale into the broadcast matmul.  Emitted after the
                    # first big x DMA trigger; the matmul needs it ~2.5us
                    # later than that.
                    ones = cpool.tile([G, P], BF16, name="ones")
                    nc.gpsimd.memset(ones, SCALE)

            # wp_ps[m, n] = sum_g ones[g, m] * ws[g, n] = SCALE*w_total[n]
            # on every partition m.  N<=512 per matmul (one PSUM bank each).
            wp_ps = ppool.tile([P, KB], F32)
            for j in range(KB // 512):
                nc.tensor.matmul(
                    wp_ps[:, j * 512 : (j + 1) * 512],
                    ones,
                    ws[:, j * 512 : (j + 1) * 512],
                    start=True,
                    stop=True,
                )
            # PSUM f32 -> SBUF bf16 so the tensor_tensor runs in the DVE
            # 2x packed mode (PSUM operands force 1x).  On ScalarE: it is
            # idle until the first accumulation (~4us later) and sits
            # closer to PSUM, while DVE's budget is the tighter one.
            wp = cpool.tile([P, KB], BF16)
            nc.scalar.activation(wp, wp_ps, mybir.ActivationFunctionType.Copy)

            acc = cpool.tile([P, NCOL], F32)

            for pi, (names, blocks, k0, kw, cols) in enumerate(pieces):
                nrb = len(blocks)
                xt = xts[pi]
                yt = cpool.tile(
                    [P, nrb * kw], BF16, tag=f"yt{pi}", name=f"yt{pi}"
                )
                if nrb == 1:
                    nc.vector.tensor_tensor(
                        yt, xt, wp[:, k0 : k0 + kw], op=mybir.AluOpType.mult
                    )
                else:
                    # One fused multiply over nrb row-blocks; wp is repeated
                    # along a stride-0 middle dim.
                    x3 = xt[:].rearrange("p (a k) -> p a k", a=nrb)
                    y3 = yt[:].rearrange("p (a k) -> p a k", a=nrb)
                    wb = (
                        wp[:, k0 : k0 + kw]
                        .unsqueeze(1)
                        .broadcast_to([P, nrb, kw])
                    )
                    nc.vector.tensor_tensor(y3, x3, wb, op=mybir.AluOpType.mult)
                seg = (nrb * kw) // len(names)
                for a, name in enumerate(names):
                    accumulate(
                        name,
                        yt[:, a * seg : (a + 1) * seg],
                        acc[:, cols[a] : cols[a] + 1],
                    )
                if pi == 6:
                    # Columns 0..7 (blk1-7) are final once the blk7 halves
                    # are accumulated: store them while block 0 is still
                    # in flight; only the 2-column block-0 store plus its
                    # completion receipt trail the final reduce.
                    nc.sync.dma_start(
                        out=out.ap()[:, 0 : NCOL - 2], in_=acc[:, 0 : NCOL - 2]
                    )
            nc.sync.dma_start(
                out=out.ap()[:, NCOL - 2 : NCOL], in_=acc[:, NCOL - 2 : NCOL]
            )
    return nc


def kernel(x: np.ndarray, wsums: np.ndarray) -> np.ndarray:
    global _built, LAST_RESULTS
    if _built is None:
        _built = _build()
    nc = _built

    x = np.ascontiguousarray(np.asarray(x, dtype=np.float32))
    wsums = np.ascontiguousarray(np.asarray(wsums, dtype=np.float32))

    in_maps = []
    for c in range(N_CORES):
        bb_i, kb_i = divmod(c, KSHARD)
        xs = np.ascontiguousarray(
            x[bb_i * BB : (bb_i + 1) * BB, kb_i * KB : (kb_i + 1) * KB]
        )
        wsl = np.ascontiguousarray(wsums[:, kb_i * KB : (kb_i + 1) * KB])
        in_maps.append({"x_shard": xs, "wsums_shard": wsl})

    res = bass_utils.run_bass_kernel_spmd(
        nc,
        in_maps,
        core_ids=list(range(N_CORES)),
        trace=TRACE,
        **TRACE_KWARGS,
    )
    LAST_RESULTS = res

    parts = []
    for bb_i in range(BSHARD):
        tot = None
        for kb_i in range(KSHARD):
            acc = res.results[bb_i * KSHARD + kb_i]["out_acc"]  # [P, NCOL]
            # cols 0..5 = blk1..6; blk7 = col6+col7; blk0 = col8+col9.
            blk = np.concatenate(
                [
                    (acc[:, 8] + acc[:, 9])[:, None],
                    acc[:, 0:6],
                    (acc[:, 6] + acc[:, 7])[:, None],
                ],
                axis=1,
            )  # [P, NBLK] in block order 0..7
            vec = blk.T.reshape(BB)  # row 128*j + p  <-  blk[p, j]
            tot = vec if tot is None else tot + vec
        parts.append(tot)
    return np.concatenate(parts).astype(np.float32)[:, None]
